# revision 8
# baseline (speedup 1.0000x reference)
"""Trainium2 Bass kernel for nn_CRF (gnn_message_passing).

Reference computation (per batch b of 256):
    sim   = (F F^T) / (|f_n||f_m|)        F = feats[b]  [N=256, E=512]
    P     = sim * W_sym                   W_sym = (W + W^T)/2  [N, N]
    lg_0  = logits[b]                     [N, 1]
    10x:  lg = logits + P @ (2*sigmoid(lg) - 1)     (2*sig(x)-1 == tanh(x/2))

Strategy: pure data parallel, 32 batches per core on 8 NeuronCores.
Per core we build row-scaled potentials Abar = diag(r) * (F F^T) * W_sym in
SBUF as bf16 (r = 1/|f| from a PE ones-matvec over squared features), then run
the 10 CRF iterations fully on-chip:
    v = tanh(0.5 * lg)      e~ = Abar^T v  (PE, 128 small matvecs)
    lg = logits + r * e~    (the row scale of Abar absorbs the other r factor)
Iteration state lives in a dense [128, 64] layout:
partition p = n mod 128, column = 32*(b div 16) + 16*(n div 128) + (b mod 16).
The batch halves (G = b div 16) are pipelined so VectorE/ScalarE work of one
half overlaps TensorE matvecs of the other.

feats are uploaded bf16-cast + [E, BP*N]-transposed from host (layout prep
only), so the device issues only large contiguous DMAs.
"""

import sys

sys.path.insert(0, "/opt/trn_rl_repo")

from contextlib import ExitStack

import ml_dtypes
import numpy as np

import concourse.bacc as bacc
import concourse.mybir as mybir
import concourse.tile as tile
from concourse.bass_utils import run_bass_kernel_spmd

B, N, E, ITER = 256, 256, 512, 10
NCORES = 8
BP = B // NCORES  # 32 batches per core
P = 128  # partitions
NH = N // P  # 2 halves of the node dim
EC = E // P  # 4 chunks of the embedding dim
NG = 8  # DMA batch groups
GB = BP // NG  # 4 batches per group
COLS = NH * BP  # 64 columns of iteration-state layout
PG = 2  # pipeline groups over batches
PGB = BP // PG  # 16 batches per pipeline group
GCOLS = COLS // PG  # 32 columns per pipeline group

F32 = mybir.dt.float32
BF16 = mybir.dt.bfloat16
AF = mybir.ActivationFunctionType
MULT = mybir.AluOpType.mult

_CACHE: dict = {}


def _col(b, h):
    g, lb = divmod(b, PGB)
    return GCOLS * g + PGB * h + lb


def _build_nc():
    nc = bacc.Bacc(
        "TRN2",
        target_bir_lowering=False,
        debug=False,
        enable_asserts=False,
        num_devices=NCORES,
    )

    ftT = nc.dram_tensor("ftT", [E, BP * N], BF16, kind="ExternalInput").ap()
    logT = nc.dram_tensor("logT", [P, COLS], F32, kind="ExternalInput").ap()
    wsym = nc.dram_tensor("wsym", [N, N], F32, kind="ExternalInput").ap()
    outT = nc.dram_tensor("outT", [P, COLS], F32, kind="ExternalOutput").ap()

    with tile.TileContext(nc) as tc, ExitStack() as ctx:
        cpool = ctx.enter_context(tc.tile_pool(name="cpool", bufs=1))
        ftp_pool = ctx.enter_context(tc.tile_pool(name="ftp", bufs=1))
        sq_pool = ctx.enter_context(tc.tile_pool(name="sq", bufs=10))
        a_pool = ctx.enter_context(tc.tile_pool(name="apool", bufs=1))
        it_pool = ctx.enter_context(tc.tile_pool(name="itpool", bufs=2))

        # ---- tiles ----
        # ftp[c][g][p, j*N + n] = feats[b = g*GB + j, n, c*128 + p]
        ftp = [
            [
                ftp_pool.tile([P, GB * N], BF16, tag=f"ftp{c}_{g}", name=f"ftp{c}_{g}")
                for g in range(NG)
            ]
            for c in range(EC)
        ]
        a_tiles = [
            [a_pool.tile([P, N], BF16, tag=f"A{b}_{h}", name=f"A{b}_{h}") for h in range(NH)]
            for b in range(BP)
        ]
        w_sb = [cpool.tile([P, N], F32, tag=f"wsym{h}", name=f"wsym{h}") for h in range(NH)]
        logT_sb = cpool.tile([P, COLS], F32, tag="logT", name="logT_sb")
        ones = cpool.tile([P, 1], BF16, tag="ones", name="ones")
        ns = cpool.tile([P, COLS], F32, tag="ns", name="ns")
        r = cpool.tile([P, COLS], F32, tag="r", name="r")
        out_sb = cpool.tile([P, COLS], F32, tag="out_sb", name="out_sb")

        # r viewed [p, (G h) lb] for per-DMA-group slices
        r4 = r[:].rearrange("p (x l) -> p x l", l=PGB)
        ns4 = ns[:].rearrange("p (x l) -> p x l", l=PGB)

        # ---- DMA for first group, then constants, then remaining groups ----
        for c in range(EC):
            nc.sync.dma_start(ftp[c][0][:], ftT[c * P : (c + 1) * P, 0 : GB * N])
        for h in range(NH):
            nc.sync.dma_start(w_sb[h][:], wsym[h * P : (h + 1) * P, :])
        nc.sync.dma_start(logT_sb[:], logT)
        nc.vector.memset(ones[:], 1.0)
        for g in range(1, NG):
            for c in range(EC):
                nc.sync.dma_start(
                    ftp[c][g][:],
                    ftT[c * P : (c + 1) * P, g * GB * N : (g + 1) * GB * N],
                )

        with tc.tile_pool(name="psumD", bufs=4, space="PSUM") as psumD, tc.tile_pool(
            name="psumN", bufs=1, space="PSUM"
        ) as psumN:
            # n2 pair-tiles: groups (2k, 2k+1) share one bank;
            # col_local = 8*h + (b - 8k)
            n2p = [
                psumN.tile([P, 2 * GB * NH], F32, tag=f"n2_{k}", name=f"n2_{k}")
                for k in range(NG // 2)
            ]
            for g in range(NG):
                k, odd = divmod(g, 2)
                sq = []
                for c in range(EC):
                    s = sq_pool.tile([P, GB * N], BF16, name="sq")
                    if (g * EC + c) % 3 == 0:
                        nc.vector.tensor_mul(s[:], ftp[c][g][:], ftp[c][g][:])
                    else:
                        nc.scalar.activation(s[:], ftp[c][g][:], AF.Square)
                    sq.append(s)

                pDs = []
                for j in range(GB):
                    b = g * GB + j
                    pD = psumD.tile([P, NH * N], F32, name="pD")
                    pDs.append(pD)
                    for h in range(NH):
                        for c in range(EC):
                            nc.tensor.matmul(
                                pD[:, h * N : (h + 1) * N],
                                ftp[c][g][:, j * N + h * P : j * N + (h + 1) * P],
                                ftp[c][g][:, j * N : (j + 1) * N],
                                start=(c == 0),
                                stop=(c == EC - 1),
                            )
                    for h in range(NH):
                        cl = 2 * GB * h + odd * GB + j
                        for c in range(EC):
                            nc.tensor.matmul(
                                n2p[k][:, cl : cl + 1],
                                sq[c][:, j * N + h * P : j * N + (h + 1) * P],
                                ones[:],
                                start=(c == 0),
                                stop=(c == EC - 1),
                            )
                # r for group g, then the deferred A-assembly
                Gg, base = divmod(g * GB, PGB)
                n2v = n2p[k][:].rearrange("p (h j) -> p h j", h=NH)[
                    :, :, odd * GB : (odd + 1) * GB
                ]
                nc.scalar.activation(
                    ns4[:, 2 * Gg : 2 * Gg + 2, base : base + GB], n2v, AF.Sqrt
                )
                nc.vector.reciprocal(
                    r4[:, 2 * Gg : 2 * Gg + 2, base : base + GB],
                    ns4[:, 2 * Gg : 2 * Gg + 2, base : base + GB],
                )
                for jj in range(GB):
                    bb = g * GB + jj
                    for h in range(NH):
                        nc.vector.scalar_tensor_tensor(
                            a_tiles[bb][h][:],
                            pDs[jj][:, h * N : (h + 1) * N],
                            r[:, _col(bb, h) : _col(bb, h) + 1],
                            w_sb[h][:],
                            MULT,
                            MULT,
                        )

        # ---- CRF iterations, pipelined over PG batch groups ----
        with tc.tile_pool(name="psumE", bufs=2, space="PSUM") as psumE:
            vs = []
            for g in range(PG):
                v0 = it_pool.tile([P, GCOLS], BF16, tag=f"v{g}", name=f"v{g}")
                nc.scalar.activation(
                    v0[:], logT_sb[:, GCOLS * g : GCOLS * (g + 1)], AF.Tanh, scale=0.5
                )
                vs.append(v0)

            def rsl(t_, g_):
                return t_[:, GCOLS * g_ : GCOLS * (g_ + 1)]

            for t in range(ITER):
                pEs = []
                for g in range(PG):
                    pE = psumE.tile([P, GCOLS], F32, name=f"pE{g}", tag=f"pE{g}")
                    for lb in range(PGB):
                        b = g * PGB + lb
                        for h in range(NH):
                            for hp in range(NH):
                                nc.tensor.matmul(
                                    pE[:, PGB * h + lb : PGB * h + lb + 1],
                                    a_tiles[b][hp][:, h * P : (h + 1) * P],
                                    vs[g][:, PGB * hp + lb : PGB * hp + lb + 1],
                                    start=(hp == 0),
                                    stop=(hp == NH - 1),
                                )
                    pEs.append(pE)
                for g in range(PG):
                    if t < ITER - 1:
                        er = it_pool.tile([P, GCOLS], F32, tag=f"er{g}", name=f"er{g}")
                        nc.vector.tensor_mul(er[:], pEs[g][:], rsl(r, g))
                        lg = it_pool.tile([P, GCOLS], F32, tag=f"lg{g}", name=f"lg{g}")
                        nc.vector.tensor_add(lg[:], er[:], rsl(logT_sb, g))
                        vnew = it_pool.tile([P, GCOLS], BF16, tag=f"v{g}", name=f"v{g}")
                        nc.scalar.activation(vnew[:], lg[:], AF.Tanh, scale=0.5)
                        vs[g] = vnew
                    else:
                        er = it_pool.tile([P, GCOLS], F32, tag=f"er{g}", name=f"er{g}")
                        nc.vector.tensor_mul(er[:], pEs[g][:], rsl(r, g))
                        nc.vector.tensor_add(rsl(out_sb, g), er[:], rsl(logT_sb, g))
            nc.sync.dma_start(outT, out_sb[:])

    nc.compile()
    return nc


def _get_nc():
    if "nc" not in _CACHE:
        _CACHE["nc"] = _build_nc()
    return _CACHE["nc"]


# host-side index map: column <-> (batch, half)
_COLMAP = np.empty(COLS, dtype=np.int64)  # col -> b*NH + h
for _b in range(BP):
    for _h in range(NH):
        _COLMAP[_col(_b, _h)] = _b * NH + _h


def _make_in_maps(feats, logits, W):
    wsym = ((W[0] + W[0].T) * 0.5).astype(np.float32)
    in_maps = []
    for i in range(NCORES):
        fs = feats[i * BP : (i + 1) * BP].reshape(BP * N, E)
        ftT = np.ascontiguousarray(fs.T).astype(ml_dtypes.bfloat16)
        lg = logits[i * BP : (i + 1) * BP, :, 0].astype(np.float32)
        lgh = lg.reshape(BP, NH, P)  # [b, h, p]
        lgT = np.ascontiguousarray(lgh[_COLMAP // NH, _COLMAP % NH, :].T)
        in_maps.append({"ftT": ftT, "logT": lgT, "wsym": wsym})
    return in_maps


def _unshard(results):
    outs = []
    for i in range(NCORES):
        oT = np.asarray(results[i]["outT"], dtype=np.float32)  # [P, COLS]
        oc = np.empty((BP, NH, P), dtype=np.float32)
        oc[_COLMAP // NH, _COLMAP % NH, :] = oT.T
        outs.append(oc.reshape(BP, N))
    return np.concatenate(outs, axis=0).reshape(B, N, 1).astype(np.float32)


def run(feats, logits, W, trace=False, **kwargs):
    nc = _get_nc()
    in_maps = _make_in_maps(np.asarray(feats), np.asarray(logits), np.asarray(W))
    res = run_bass_kernel_spmd(
        nc, in_maps, core_ids=list(range(NCORES)), trace=trace, **kwargs
    )
    return _unshard(res.results), res


def kernel(feats, logits, W):
    out, _ = run(feats, logits, W)
    return out
